# revision 1
# baseline (speedup 1.0000x reference)
"""Trainium2 Bass kernel for AllAtomEnergyBranch (3-layer MLP over broadcast concat).

Math (per batch b, position n, edge e):
    out[b,n,e,0] = W3^T relu(W2^T relu(Wh^T h[b,n] + We^T e_feat[e] + b1) + b2) + b3

Sharding: data-parallel over B (8 batches -> 8 NeuronCores), weights replicated.
Each core computes its [64, 256] output slice independently; no collectives.

Per-core dataflow (k-major layouts so the PE contracts over partitions):
  - epT [512k, 256e] = (We_aug.T @ eT_aug) with b1 folded in via an augmented
    ones-row (K=65), stored bf16 in SBUF.
  - hpT [512k, 64n]  = Wh.T @ hT, stored f32 in SBUF.
  - per block of 2 n's (32 blocks):
      X^T[kt] [128,512] = relu(epT[kt] + hpT[kt][:,n])   (DVE tensor_scalar add+max)
      psumY[jt] = sum_kt W2[kt,jt].T @ X^T[kt]           (PE, bf16)
      Y^T[jt] = relu(psumY[jt] + b2[jt])                 (ACT, per-partition bias)
      Z^T[jt] = Y^T[jt] * w3[jt]  (signed, post-relu)    (DVE per-partition mul)
      ssum    = sum_jt Z^T[jt]    (3 adds)               (DVE)
      psumO   = ones.T @ ssum     (partition reduce)     (PE, single matmul)
      out_blk = psumO + b3                               (ACT Identity + bias)

Performance (measured on TRN2 silicon, For_i-loop delta-R method):
  - cost model (TimelineSim): 131.1 us single-shot
  - hardware: ~47-55 us/iter cool-burst, 152-169 us/iter sustained-throttle
    (identical NEFF; the spread is chip power/thermal state, not structure)
  - per-MM 512-col bf16 stream measured 167.5 ns cool / ~208 ns sustained,
    with LDWEIGHTS fully hidden in this alternating-weights pattern
    (same-stationary grouping measures SLOWER: 233 ns/MM)
  - at sustained state the single-engine rates for one iteration's work are
    PE ~108 us / ACT ~116 us / DVE ~119 us; the kernel runs ~160 us there
    and is insensitive (±2%) to pool depth, out-op engine, ACT op pairing,
    or fold restructuring.  Extra narrow reduce-matmuls are heavily
    penalized (+3 MMs/block -> +37 us), so the single ones-matmul
    partition-reduce per block is kept.
  - alternative folds implemented for measurement (build flags): 'pe4'
    (PE-native W3 fold, +37us), 'sig' (|w3| folded into ACT scale +
    sign-consistent hidden permutation, +2 reduce-MMs), 'act2' (paired
    2-bank PSUM + [128,1024] ACT relu, tie), gps_adds (fold adds on the
    idle GPSIMD, +28%), dma_batch (4-way batched output DMA, tie), plus
    pool/bank-depth sweeps (all ties).  All verified correct on silicon;
    none beat this default.
  - rel err vs f32 reference: 0.0049 (harness gate 2e-2).
"""

import numpy as np
import ml_dtypes

import concourse.bass as bass
import concourse.mybir as mybir
from concourse import bacc
from concourse.bass import ts
from concourse.tile import TileContext
from concourse.bass_utils import run_bass_kernel_spmd

BF16 = mybir.dt.bfloat16
F32 = mybir.dt.float32

B, N, H = 8, 64, 256
NE, E = 256, 64
HID, OUT = 512, 1
KT = HID // 128   # 4 k-tiles of layer-1 output / layer-2 contraction
JT = HID // 128   # 4 j-tiles of layer-2 output / layer-3 contraction
HT = H // 128     # 2 h-tiles of layer-1 contraction
NBLK = N // 2     # blocks of 2 n-values -> 512 moving columns per matmul


def build(nc, repeat=1, dyn_repeat=None, y_bufs=6, x_bufs=3, yp_bufs=3,
          mm3_dve=1, o_bufs=2, ones_ct=0, stage="full", xconst=False,
          opool_bufs=4, out_eng="act", fold=None, act2=False, dve_all=False,
          dma_batch=1, gps_adds=False):
    """fold: None -> derive from mm3_dve ('dve' or 'pe4'); 'sig' -> |w3|
    folded into the ACT relu scale, signs folded into the reduce-matmul
    stationaries (1 pair + 2 singleton tiles; see make_in_maps).
    act2: pair the two j-tiles of a 2-bank PSUM tile and relu both with a
    single [128,1024] ACT op (requires b2[j] equal within each pair, e.g.
    b2 == 0)."""
    """Build the per-core graph. All 8 cores run this same program.

    repeat / dyn_repeat: repeat the whole computation inside the NEFF
    (python-unrolled / For_i hardware loop) — used only for benchmarking.
    """
    ht_d = nc.declare_dram_parameter("ht", [HT, 128, N], BF16, isOutput=False)
    wh_d = nc.declare_dram_parameter("wh", [HT, 128, HID], BF16, isOutput=False)
    we_d = nc.declare_dram_parameter("we", [E + 1, HID], BF16, isOutput=False)
    et_d = nc.declare_dram_parameter("et", [E + 1, NE], BF16, isOutput=False)
    w2_d = nc.declare_dram_parameter("w2", [KT, 128, HID], BF16, isOutput=False)
    b2_d = nc.declare_dram_parameter("b2", [128, JT], F32, isOutput=False)
    w3_d = nc.declare_dram_parameter("w3", [128, JT], BF16, isOutput=False)
    w3f_d = nc.declare_dram_parameter("w3f", [128, JT], F32, isOutput=False)
    b3_d = nc.declare_dram_parameter("b3", [128, 1], F32, isOutput=False)
    out_d = nc.declare_dram_parameter(
        "out", [NBLK // dma_batch, 512 * dma_batch], F32, isOutput=True)
    outp_d = None
    if stage == "mm2relu":
        outp_d = nc.declare_dram_parameter("outp", [NBLK, 512], BF16,
                                           isOutput=True)
    if fold is None:
        fold = "dve" if mm3_dve else "pe4"
    w2s_d = sc_d = b2s_d = vsta_d = None
    if fold == "sig":
        w2s_d = nc.declare_dram_parameter("w2s", [KT, 128, HID], BF16,
                                          isOutput=False)
        sc_d = nc.declare_dram_parameter("sc", [128, JT], F32, isOutput=False)
        b2s_d = nc.declare_dram_parameter("b2s", [128, JT], F32,
                                          isOutput=False)
        vsta_d = nc.declare_dram_parameter("vsta", [128, JT], BF16,
                                           isOutput=False)

    relu = mybir.ActivationFunctionType.Relu
    ident = mybir.ActivationFunctionType.Identity
    add = mybir.AluOpType.add
    mult = mybir.AluOpType.mult
    amax = mybir.AluOpType.max

    with TileContext(nc) as tc:
        with (
            tc.tile_pool(name="const", bufs=1) as cpool,
            tc.tile_pool(name="xp", bufs=x_bufs) as xpool,
            tc.tile_pool(name="yp", bufs=yp_bufs) as ypool,
            tc.tile_pool(name="op", bufs=opool_bufs) as opool,
            tc.tile_pool(name="psY", bufs=y_bufs, space="PSUM") as y_ps,
            tc.tile_pool(name="psO", bufs=o_bufs, space="PSUM") as o_ps,
        ):
            # ---- load weights / inputs into SBUF ----
            # Order matters: everything the preamble matmuls need (ht/we/et/wh)
            # goes first so the PE can start while W2 is still in flight.
            we_t = cpool.tile([E + 1, HID], BF16, tag="we")
            nc.sync.dma_start(out=we_t[:], in_=we_d[:])
            et_t = cpool.tile([E + 1, NE], BF16, tag="et")
            nc.sync.dma_start(out=et_t[:], in_=et_d[:])
            ht_t = []
            for h in range(HT):
                t = cpool.tile([128, N], BF16, tag=f"ht{h}", name=f"ht{h}")
                nc.sync.dma_start(out=t[:], in_=ht_d[h])
                ht_t.append(t)
            wh_t = []
            for h in range(HT):
                t = cpool.tile([128, HID], BF16, tag=f"wh{h}", name=f"wh{h}")
                nc.sync.dma_start(out=t[:], in_=wh_d[h])
                wh_t.append(t)
            b2_t = cpool.tile([128, JT], F32, tag="b2")
            nc.sync.dma_start(out=b2_t[:], in_=b2_d[:])
            w3_t = cpool.tile([128, JT], BF16, tag="w3")
            nc.sync.dma_start(out=w3_t[:], in_=w3_d[:])
            w3f_t = cpool.tile([128, JT], F32, tag="w3f")
            nc.sync.dma_start(out=w3f_t[:], in_=w3f_d[:])
            ones_t = cpool.tile([128, 1], BF16, tag="ones")
            nc.vector.memset(ones_t[:], 1.0)
            b3_t = cpool.tile([128, 1], F32, tag="b3")
            nc.sync.dma_start(out=b3_t[:], in_=b3_d[:])
            w2_t = []
            for k in range(KT):
                t = cpool.tile([128, HID], BF16, tag=f"w2{k}", name=f"w2{k}")
                nc.sync.dma_start(out=t[:], in_=(w2s_d if fold == "sig"
                                                 else w2_d)[k])
                w2_t.append(t)
            sc_t = b2s_t = vsta_t = None
            if fold == "sig":
                sc_t = cpool.tile([128, JT], F32, tag="sc")
                nc.sync.dma_start(out=sc_t[:], in_=sc_d[:])
                b2s_t = cpool.tile([128, JT], F32, tag="b2s")
                nc.sync.dma_start(out=b2s_t[:], in_=b2s_d[:])
                vsta_t = cpool.tile([128, JT], BF16, tag="vsta")
                nc.sync.dma_start(out=vsta_t[:], in_=vsta_d[:])

            ep_t = [cpool.tile([128, NE], BF16, tag=f"ep{k}", name=f"ep{k}") for k in range(KT)]
            hp_t = [cpool.tile([128, N], F32, tag=f"hp{k}", name=f"hp{k}") for k in range(KT)]

            # PE warm-up: dependency-free matmuls on memset data issue
            # immediately (while the weight DMAs are still in flight) so the
            # HAM clock-gate reaches 8/8 before the first real matmul.  Runs
            # once per NEFF, outside any repeat loop; result is never read.
            warm_t = cpool.tile([128, 512], BF16, tag="warm")
            nc.vector.memset(warm_t[:], 0.5)
            for w in range(8):
                if act2:
                    psw = y_ps.tile([128, 1024], F32, tag="Y2", name="psW",
                                    bufs=3)[:, 0:512]
                else:
                    psw = y_ps.tile([128, 512], F32, tag="Y", name="psW")[:]
                nc.tensor.matmul(
                    psw, warm_t[:, 0:128], warm_t[:],
                    start=True, stop=True,
                )

            def psum_pre(name):
                if act2:
                    t = y_ps.tile([128, 1024], F32, tag="Y2", name=name,
                                  bufs=3)
                    return t
                return None

            def body():
                # ---- preamble: epT (with b1 via aug row) and hpT ----
                for k in range(KT):
                    if act2:
                        ps = psum_pre("psE")[:, 0:NE]
                    else:
                        ps = y_ps.tile([128, NE], F32, tag="Y", name="psE")[:]
                    nc.tensor.matmul(
                        ps, we_t[:, ts(k, 128)], et_t[:], start=True, stop=True
                    )
                    nc.vector.tensor_copy(out=ep_t[k][:], in_=ps)
                for k in range(KT):
                    if act2:
                        ps = psum_pre("psH")[:, 0:N]
                    else:
                        ps = y_ps.tile([128, N], F32, tag="Y", name="psH")[:]
                    for h in range(HT):
                        nc.tensor.matmul(
                            ps,
                            wh_t[h][:, ts(k, 128)],
                            ht_t[h][:],
                            start=(h == 0),
                            stop=(h == HT - 1),
                        )
                    nc.vector.tensor_copy(out=hp_t[k][:], in_=ps)

                # ---- main loop over blocks of 2 n-values ----
                xc = None
                if xconst:
                    xc = []
                    for k in range(KT):
                        t = cpool.tile([128, 512], BF16, tag=f"xc{k}",
                                       name=f"xc{k}")
                        for j in range(2):
                            nc.vector.tensor_scalar(
                                out=t[:, ts(j, NE)],
                                in0=ep_t[k][:],
                                scalar1=hp_t[k][:, j : j + 1],
                                scalar2=0.0,
                                op0=add,
                                op1=amax,
                            )
                        xc.append(t)
                ssum_q = []   # (blk, ssum) pending partition-reduce (ones_ct)
                for blk in range(NBLK):
                    if xconst:
                        xt = xc
                    else:
                        xt = []
                        for k in range(KT):
                            t = xpool.tile([128, 512], BF16, tag=f"x{k}", name=f"x{k}")
                            for j in range(2):
                                n = 2 * blk + j
                                nc.vector.tensor_scalar(
                                    out=t[:, ts(j, NE)],
                                    in0=ep_t[k][:],
                                    scalar1=hp_t[k][:, n : n + 1],
                                    scalar2=0.0,
                                    op0=add,
                                    op1=amax,
                                )
                            xt.append(t)

                    blk_dve_pre = mm3_dve and blk < NBLK - 1
                    if ones_ct and blk_dve_pre:
                        pso = None
                    elif ones_ct:
                        psoT = o_ps.tile([128, 512], F32, tag="po4", name="po4")
                        pso = psoT[0:1, :]
                    else:
                        pso = o_ps.tile([1, 512], F32, tag="po")
                    zts = []
                    yts = []
                    # last block: PE-native W3 matmuls (interleave with its own
                    # mm2s) so the kernel tail doesn't wait on the DVE fold
                    blk_dve = ((fold == "dve") and not act2
                               and (dve_all or blk < NBLK - 1))
                    yt_last = None
                    if act2:
                        assert stage == "full" and fold == "dve"
                        for jp in range(2):
                            psy2 = y_ps.tile([128, 1024], F32, tag="Y2",
                                             name="Y2", bufs=3)
                            for jj in range(2):
                                j = 2 * jp + jj
                                for k in range(KT):
                                    nc.tensor.matmul(
                                        psy2[:, 512 * jj : 512 * (jj + 1)],
                                        w2_t[k][:, ts(j, 128)],
                                        xt[k][:],
                                        start=(k == 0),
                                        stop=(k == KT - 1),
                                    )
                            yt2 = ypool.tile([128, 1024], BF16, tag=f"yp{jp}",
                                             name=f"yp{jp}")
                            nc.scalar.activation(
                                out=yt2[:],
                                in_=psy2[:],
                                func=relu,
                                bias=b2_t[:, 2 * jp : 2 * jp + 1],
                                scale=1.0,
                            )
                            for jj in range(2):
                                j = 2 * jp + jj
                                zt = ypool.tile([128, 512], BF16, tag=f"z{j}",
                                                name=f"z{j}")
                                nc.vector.tensor_scalar(
                                    out=zt[:],
                                    in0=yt2[:, 512 * jj : 512 * (jj + 1)],
                                    scalar1=w3f_t[:, j : j + 1],
                                    scalar2=None,
                                    op0=mult,
                                )
                                zts.append(zt)
                        s01 = ypool.tile([128, 512], BF16, tag="s01", name="s01")
                        nc.vector.tensor_add(out=s01[:], in0=zts[0][:], in1=zts[1][:])
                        s23 = ypool.tile([128, 512], BF16, tag="s23", name="s23")
                        nc.vector.tensor_add(out=s23[:], in0=zts[2][:], in1=zts[3][:])
                        ssum = ypool.tile([128, 512], BF16, tag="ss", name="ss")
                        nc.vector.tensor_add(out=ssum[:], in0=s01[:], in1=s23[:])
                        nc.tensor.matmul(
                            pso[:], ones_t[:], ssum[:], start=True, stop=True
                        )
                        ot = opool.tile([1, 512], F32, tag="o")
                        if out_eng == "dve" or (out_eng == "alt" and blk % 2):
                            nc.vector.tensor_scalar(
                                out=ot[:], in0=pso[:], scalar1=b3_t[0:1, 0:1],
                                scalar2=None, op0=add)
                        else:
                            nc.scalar.activation(
                                out=ot[:], in_=pso[:], func=ident,
                                bias=b3_t[0:1, 0:1], scale=1.0)
                        nc.sync.dma_start(out=out_d[blk : blk + 1, :], in_=ot[:])
                        continue
                    for j in range(JT):
                        psy = y_ps.tile([128, 512], F32, tag="Y")
                        for k in range(KT):
                            nc.tensor.matmul(
                                psy[:],
                                w2_t[k][:, ts(j, 128)],
                                xt[k][:],
                                start=(k == 0),
                                stop=(k == KT - 1),
                            )
                        yt = ypool.tile([128, 512], BF16, tag=f"y{j}", name=f"y{j}")
                        nc.scalar.activation(
                            out=yt[:],
                            in_=psy[:],
                            func=relu,
                            bias=(b2s_t[:, j : j + 1] if fold == "sig"
                                  else b2_t[:, j : j + 1]),
                            scale=(sc_t[:, j : j + 1] if fold == "sig" and j < 2
                                   else 1.0),
                        )
                        yt_last = yt
                        yts.append(yt)
                        if stage == "mm2relu" or fold == "sig":
                            continue
                        if blk_dve:
                            # fold signed w3 now (y >= 0 post-relu, so a plain
                            # per-partition multiply is exact w3*relu(.))
                            zt = ypool.tile([128, 512], BF16,
                                            tag=f"z{j}", name=f"z{j}")
                            nc.vector.tensor_scalar(
                                out=zt[:],
                                in0=yt[:],
                                scalar1=w3f_t[:, j : j + 1],
                                scalar2=None,
                                op0=mult,
                            )
                            zts.append(zt)
                        else:
                            nc.tensor.matmul(
                                pso[:],
                                w3_t[:, j : j + 1],
                                yt[:],
                                start=(j == 0),
                                stop=(j == JT - 1),
                            )
                    if stage == "mm2relu":
                        nc.sync.dma_start(out=outp_d[blk : blk + 1, :],
                                          in_=yt_last[0:1, :])
                        continue
                    if fold == "sig":
                        ssA = ypool.tile([128, 512], BF16, tag="ssA",
                                         name="ssA")
                        nc.vector.tensor_add(out=ssA[:], in0=yts[0][:],
                                             in1=yts[1][:])
                        nc.tensor.matmul(pso[:], vsta_t[:, 0:1], ssA[:],
                                         start=True, stop=False)
                        nc.tensor.matmul(pso[:], vsta_t[:, 2:3], yts[2][:],
                                         start=False, stop=False)
                        nc.tensor.matmul(pso[:], vsta_t[:, 3:4], yts[3][:],
                                         start=False, stop=True)
                    elif blk_dve:
                        adder = nc.gpsimd if gps_adds else nc.vector
                        s01 = ypool.tile([128, 512], BF16, tag="s01", name="s01")
                        adder.tensor_add(out=s01[:], in0=zts[0][:], in1=zts[1][:])
                        s23 = ypool.tile([128, 512], BF16, tag="s23", name="s23")
                        adder.tensor_add(out=s23[:], in0=zts[2][:], in1=zts[3][:])
                        ssum = ypool.tile([128, 512], BF16, tag="ss", name="ss",
                                          bufs=6 if ones_ct else None)
                        adder.tensor_add(out=ssum[:], in0=s01[:], in1=s23[:])
                        if ones_ct:
                            # batch ones_ct blocks; col-tiled ones-matmuls go
                            # to different array column groups of one PSUM
                            # bank, then a single ACT op drains all strips
                            ssum_q.append((blk, ssum))
                            if len(ssum_q) == ones_ct:
                                pso4 = o_ps.tile([128, 512], F32, tag="po4",
                                                 name="po4")
                                for bi, (b_, ss_) in enumerate(ssum_q):
                                    nc.tensor.matmul(
                                        pso4[32 * bi : 32 * bi + 1, :],
                                        ones_t[:],
                                        ss_[:],
                                        start=True,
                                        stop=True,
                                        tile_position=(0, 32 * bi),
                                    )
                                hi = 32 * (len(ssum_q) - 1) + 1
                                otg = opool.tile([hi, 512], F32, tag="og",
                                                 name="og")
                                nc.scalar.activation(
                                    out=otg[:],
                                    in_=pso4[0:hi, :],
                                    func=ident,
                                    bias=b3_t[0:hi, :],
                                    scale=1.0,
                                )
                                for bi, (b_, ss_) in enumerate(ssum_q):
                                    nc.sync.dma_start(
                                        out=out_d[b_ : b_ + 1, :],
                                        in_=otg[32 * bi : 32 * bi + 1, :])
                                ssum_q = []
                            continue
                        nc.tensor.matmul(
                            pso[:], ones_t[:], ssum[:], start=True, stop=True
                        )
                    if dma_batch > 1:
                        if blk % dma_batch == 0:
                            staged = opool.tile([1, 512 * dma_batch], F32,
                                                tag="og2", name="og2")
                        c0 = 512 * (blk % dma_batch)
                        ot_ap = staged[:, c0 : c0 + 512]
                    else:
                        ot = opool.tile([1, 512], F32, tag="o")
                        ot_ap = ot[:]
                    if out_eng == "dve" or (out_eng == "alt" and blk % 2):
                        nc.vector.tensor_scalar(
                            out=ot_ap,
                            in0=pso[:],
                            scalar1=b3_t[0:1, 0:1],
                            scalar2=None,
                            op0=add,
                        )
                    else:
                        nc.scalar.activation(
                            out=ot_ap,
                            in_=pso[:],
                            func=ident,
                            bias=b3_t[0:1, 0:1],
                            scale=1.0,
                        )
                    if dma_batch > 1:
                        if blk % dma_batch == dma_batch - 1:
                            g = blk // dma_batch
                            nc.sync.dma_start(
                                out=out_d[g : g + 1, :],
                                in_=staged[:],
                            )
                    else:
                        nc.sync.dma_start(out=out_d[blk : blk + 1, :],
                                          in_=ot_ap)
                # flush leftover batched blocks individually
                for b_, ss_ in ssum_q:
                    psoF = o_ps.tile([128, 512], F32, tag="po4", name="po4")
                    nc.tensor.matmul(
                        psoF[0:1, :], ones_t[:], ss_[:], start=True, stop=True
                    )
                    otF = opool.tile([1, 512], F32, tag="o")
                    nc.scalar.activation(
                        out=otF[:],
                        in_=psoF[0:1, :],
                        func=ident,
                        bias=b3_t[0:1, :],
                        scale=1.0,
                    )
                    nc.sync.dma_start(out=out_d[b_ : b_ + 1, :], in_=otF[:])
                ssum_q = []

            if dyn_repeat is not None:
                hint = (mybir.EngineType.PE, mybir.EngineType.DVE,
                        mybir.EngineType.Activation)
                with tc.For_i(0, dyn_repeat, 1, hint_engines=hint):
                    body()
            else:
                for _rep in range(repeat):
                    body()
    return nc


def make_in_maps(h_all, e_feat, W1, b1, W2, b2, W3, b3):
    bf = ml_dtypes.bfloat16
    Wh = np.ascontiguousarray(W1[:H]).astype(bf).reshape(HT, 128, HID)
    We_aug = np.concatenate([W1[H:], b1[None, :]], axis=0).astype(bf)
    eT_aug = np.concatenate(
        [e_feat.T, np.ones((1, NE), np.float32)], axis=0
    ).astype(bf)
    W2k = W2.astype(bf).reshape(KT, 128, HID)
    b2c = np.ascontiguousarray(b2.reshape(JT, 128).T).astype(np.float32)
    W3c = np.ascontiguousarray(W3.reshape(JT, 128).T).astype(bf)
    W3f = np.ascontiguousarray(W3.reshape(JT, 128).T).astype(np.float32)
    b3c = np.ascontiguousarray(
        np.broadcast_to(np.asarray(b3, np.float32).reshape(1, 1), (128, 1))
    )
    # --- 'sig' fold extras: permute hidden units so w3 signs are
    # lane-consistent within tiles 0+1 (pair, +/-1 stationary); tiles 2,3
    # keep raw signed w3 stationaries (no constraint) ---
    w3v = np.asarray(W3, np.float32).reshape(-1)
    pos = list(np.nonzero(w3v > 0)[0])
    neg = list(np.nonzero(w3v <= 0)[0])
    aP = min(128, len(pos) // 2)
    pi = np.empty(HID, np.int64)
    for p in range(128):
        if p < aP:
            pi[p] = pos.pop()
            pi[128 + p] = pos.pop()
        else:
            pi[p] = neg.pop()
            pi[128 + p] = neg.pop()
    pi[256:] = pos + neg
    W2p = W2[:, pi]
    b2p = np.asarray(b2, np.float32)[pi]
    w3p = w3v[pi]
    scm = np.ones((128, JT), np.float32)
    scm[:, 0] = np.abs(w3p[0:128])
    scm[:, 1] = np.abs(w3p[128:256])
    b2sm = np.stack([b2p[j * 128 : (j + 1) * 128] for j in range(JT)],
                    axis=1).astype(np.float32)
    b2sm[:, 0] *= scm[:, 0]
    b2sm[:, 1] *= scm[:, 1]
    vsta = np.zeros((128, JT), np.float32)
    vsta[:, 0] = np.where(np.arange(128) < aP, 1.0, -1.0)
    vsta[:, 2] = w3p[256:384]
    vsta[:, 3] = w3p[384:512]
    shared = {
        "wh": Wh, "we": We_aug, "et": eT_aug, "w2": W2k,
        "b2": b2c, "w3": W3c, "w3f": W3f, "b3": b3c,
        "w2s": np.ascontiguousarray(W2p).astype(bf).reshape(KT, 128, HID),
        "sc": scm, "b2s": b2sm, "vsta": vsta.astype(bf),
    }
    in_maps = []
    for b in range(B):
        hT = np.ascontiguousarray(h_all[b].T).astype(bf).reshape(HT, 128, N)
        in_maps.append({"ht": hT, **shared})
    return in_maps


_nc_cache = {}

# production build config (chosen by on-device comparison; see docstring)
_CONFIG = {}


def _get_nc():
    if "nc" not in _nc_cache:
        nc = bacc.Bacc("TRN2", target_bir_lowering=False, debug=False, num_devices=B)
        build(nc, **_CONFIG)
        nc.compile()
        _nc_cache["nc"] = nc
    return _nc_cache["nc"]


def kernel(h_all, e_feat, W1, b1, W2, b2, W3, b3):
    h_all = np.asarray(h_all, np.float32)
    e_feat = np.asarray(e_feat, np.float32)
    W1 = np.asarray(W1, np.float32)
    b1 = np.asarray(b1, np.float32)
    W2 = np.asarray(W2, np.float32)
    b2 = np.asarray(b2, np.float32)
    W3 = np.asarray(W3, np.float32)
    b3 = np.asarray(b3, np.float32)

    nc = _get_nc()
    in_maps = make_in_maps(h_all, e_feat, W1, b1, W2, b2, W3, b3)
    res = run_bass_kernel_spmd(nc, in_maps, core_ids=list(range(B)))
    out = np.stack([res.results[i]["out"].reshape(N, NE, OUT) for i in range(B)])
    return out.astype(np.float32)

